# revision 4
# baseline (speedup 1.0000x reference)
"""Trainium2 Bass kernel for 4-layer bidirectional GRU (H=128, T=200) + MLP head.

Data-parallel: 400 sequences -> 50 per core on 8 cores. Layout: 128 partitions
= hidden unit, free dim = batch slots [fwd 50 | bwd 50].

Per scan step the critical chain is:
  2 r-gate matmuls -> sigmoid(r) -> mul(r, q+bhn) -> add(gin) -> tanh
  -> mul(n, 1-z) -> add(z*h) -> h'
Off-chain: z/n matmuls, sigmoid(-z_pre) giving (1-z) directly, z*h product,
PSUM prefills via identity matmuls (biases + gi pre-loaded into the
accumulators), input-projection precompute and its PSUM->SBUF evictions
(split in halves to fit scheduling gaps), and a tiny PE-warming matmul after
each tanh to keep the PE out of its low-power state ahead of the gate
matmuls.

gi layout per step: [r(100) | z(100) | bhn(100) | gin(100)] - r/z/gin hold
input projections incl. biases (folded in the eviction's activation bias);
the bhn columns are DMA-broadcast once per layer so a single identity matmul
prefills the n-gate PSUM group with bhh_n before Whn*h accumulates onto it.
One (128,300) PSUM tile carries three independent accumulation groups
[r|z|q], so sigmoid(r) fires as soon as the two r matmuls stop.
"""

import sys

import numpy as np

_REPO = "/opt/trn_rl_repo"
if _REPO not in sys.path:
    sys.path.insert(0, _REPO)

B, KSEQ, T = 4, 100, 200
H = 128
L = 4
OUT = 8
NCORES = 8
N = B * KSEQ
NB = N // NCORES          # 50 per core
CT = 10                   # timesteps per precompute chunk
F16 = "float16"

_CACHE = {}


def _build_program(t_len=T, nb=NB, ct=CT, num_devices=NCORES):
    import concourse.bacc as bacc
    import concourse.mybir as mybir
    import concourse.tile as tile
    from contextlib import ExitStack

    f32 = mybir.dt.float32
    f16 = mybir.dt.float16
    AF = mybir.ActivationFunctionType
    ALU = mybir.AluOpType

    nch = t_len // ct
    W = 2 * nb                  # 100
    GW = 8 * nb                 # 400: gi step block [r|z|bhn|gin]
    GW3 = 4 * nb                # 200: layer-3 gi step block

    nc = bacc.Bacc("TRN2", target_bir_lowering=False, debug=False,
                   num_devices=num_devices)

    # ---- DRAM I/O ----
    dx0f = nc.dram_tensor("x0f", (2, t_len * nb), f16, kind="ExternalInput").ap()
    dx0r = nc.dram_tensor("x0r", (2, t_len * nb), f16, kind="ExternalInput").ap()
    dw0 = nc.dram_tensor("w0", (2, 6 * H), f16, kind="ExternalInput").ap()
    dwih = nc.dram_tensor("wihT", (36, H, H), f16, kind="ExternalInput").ap()
    dwhh = nc.dram_tensor("whhT", (24, H, H), f16, kind="ExternalInput").ap()
    dbcols = nc.dram_tensor("bcols", (H, 18), f32, kind="ExternalInput").ap()
    dbhn012 = nc.dram_tensor("bhn012", (3, H, ct * W), f16,
                             kind="ExternalInput").ap()
    dbhn3 = nc.dram_tensor("bhn3", (H, ct * nb), f16,
                           kind="ExternalInput").ap()
    dbhn3b = nc.dram_tensor("bhn3b", (H, nb), f16, kind="ExternalInput").ap()
    dident = nc.dram_tensor("ident", (H, H), f16, kind="ExternalInput").ap()
    dw1 = nc.dram_tensor("w1T", (2, H, H), f16, kind="ExternalInput").ap()
    db1 = nc.dram_tensor("b1col", (H, 1), f32, kind="ExternalInput").ap()
    dw2 = nc.dram_tensor("w2T", (H, OUT), f32, kind="ExternalInput").ap()
    db2 = nc.dram_tensor("b2col", (OUT, 1), f32, kind="ExternalInput").ap()
    dout = nc.dram_tensor("out", (OUT, nb), f32, kind="ExternalOutput").ap()

    with tile.TileContext(nc) as tc, ExitStack() as ctx:
        cpool = ctx.enter_context(tc.tile_pool(name="consts", bufs=1))
        pers = ctx.enter_context(tc.tile_pool(name="pers", bufs=1))
        pgate = ctx.enter_context(tc.tile_pool(name="pgate", bufs=1,
                                               space="PSUM"))
        ppre = ctx.enter_context(tc.tile_pool(name="ppre", bufs=2, space="PSUM"))

        # ---- constants / weights ----
        w0_sb = cpool.tile([2, 6 * H], f16)
        nc.sync.dma_start(w0_sb[:], dw0)
        wih_sb = cpool.tile([H, 36 * H], f16)
        nc.sync.dma_start(wih_sb[:].rearrange("p (i c) -> p i c", c=H),
                          dwih.rearrange("i p c -> p i c"))
        whh_sb = cpool.tile([H, 24 * H], f16)
        nc.sync.dma_start(whh_sb[:].rearrange("p (i c) -> p i c", c=H),
                          dwhh.rearrange("i p c -> p i c"))
        bcols_sb = cpool.tile([H, 18], f32)
        nc.sync.dma_start(bcols_sb[:], dbcols)
        id_sb = cpool.tile([H, H], f16)
        nc.sync.dma_start(id_sb[:], dident)
        w1_sb = cpool.tile([H, 2 * H], f16)
        nc.sync.dma_start(w1_sb[:].rearrange("p (i c) -> p i c", c=H),
                          dw1.rearrange("i p c -> p i c"))
        b1_sb = cpool.tile([H, 1], f32)
        nc.sync.dma_start(b1_sb[:], db1)
        w2_sb = cpool.tile([H, OUT], f32)
        nc.sync.dma_start(w2_sb[:], dw2)
        b2_sb = cpool.tile([OUT, 1], f32)
        nc.sync.dma_start(b2_sb[:], db2)

        x0f_sb = pers.tile([2, t_len * nb], f16, tag="x0f")
        nc.sync.dma_start(x0f_sb[:], dx0f)
        x0r_sb = pers.tile([2, t_len * nb], f16, tag="x0r")
        nc.sync.dma_start(x0r_sb[:], dx0r)

        # persistent state tiles
        xA = pers.tile([H, t_len * W], f16, tag="xA")
        xB = pers.tile([H, t_len * W], f16, tag="xB")
        gis = [pers.tile([H, ct * GW], f16, tag=f"gi{i}", name=f"gi{i}")
               for i in range(3)]
        r_sb = pers.tile([H, W], f16, tag="r_sb")
        zm_sb = pers.tile([H, W], f16, tag="zm_sb")
        tmp_sb = pers.tile([H, W], f16, tag="tmp_sb")
        n2_sb = pers.tile([H, W], f16, tag="n2_sb")
        n_sb = pers.tile([H, W], f16, tag="n_sb")
        u_sb = pers.tile([H, W], f16, tag="u_sb")
        zh_sb = pers.tile([H, W], f16, tag="zh_sb")
        nzm_sb = pers.tile([H, W], f16, tag="nzm_sb")
        zeros = pers.tile([H, W], f16, tag="zeros")
        hrot = [pers.tile([H, nb], f16, tag=f"hrot{i}", name=f"hrot{i}")
                for i in range(2)]
        hb_sb = pers.tile([H, nb], f16, tag="hb_sb")
        gib = pers.tile([H, GW3], f16, tag="gib")

        nc.vector.memset(zeros[:], 0.0)

        def wih_t(l, d, g, k):  # layers 1..3
            i = (((l - 1) * 2 + d) * 3 + g) * 2 + k
            return wih_sb[:, i * H:(i + 1) * H]

        def whh_t(l, d, g):
            i = (l * 2 + d) * 3 + g
            return whh_sb[:, i * H:(i + 1) * H]

        def bcol(l, d, g):
            i = (l - 1) * 6 + d * 3 + g
            return bcols_sb[:, i:i + 1]

        # ------------- precompute pieces (emitted interleaved) -------------
        def ev_halves(ps, dst3, bias):
            """Split one eviction into two halves along the chunk dim."""
            hh = ct // 2
            out = []
            for a in range(2):
                def ev(ps=ps, dst3=dst3, bias=bias, a=a):
                    src = ps[:].rearrange("p (tl n) -> p tl n", n=nb)
                    kw = {} if bias is None else {"bias": bias}
                    nc.scalar.activation(dst3[:, a * hh:(a + 1) * hh],
                                         src[:, a * hh:(a + 1) * hh],
                                         AF.Identity, **kw)
                out.append(ev)
            return out

        def pre_pieces_l0(c, gi):
            gi3 = gi[:, 0:ct * GW].rearrange("p (tl w) -> p tl w", w=GW)
            pieces = []
            for d in range(2):
                src = x0f_sb if d == 0 else x0r_sb
                rhs = src[:, c * ct * nb:(c + 1) * ct * nb]
                for g in range(3):
                    ps = ppre.tile([H, ct * nb], f32, tag="ppre", name="ppret")
                    lhsT = w0_sb[:, (d * 3 + g) * H:(d * 3 + g + 1) * H]

                    def mm(ps=ps, lhsT=lhsT, rhs=rhs):
                        nc.tensor.matmul(ps[:], lhsT, rhs, start=True,
                                         stop=True)

                    slot = g * W if g < 2 else 3 * W
                    dst3 = gi3[:, :, slot + d * nb: slot + (d + 1) * nb]
                    pieces.append(mm)
                    pieces += ev_halves(ps, dst3, None)
            return pieces

        def pre_pieces(l, x_in, c, gi, dirs=(0, 1)):
            gw = GW if l < 3 else GW3
            gslot = W if l < 3 else nb
            gi3 = gi[:, 0:ct * gw].rearrange("p (tl w) -> p tl w", w=gw)
            x3 = x_in[:].rearrange("p (t w) -> p t w", w=W)
            s0 = c * ct
            hi = t_len - 1 - s0
            lo = hi - ct
            asc = slice(s0, s0 + ct)
            dsc = slice(hi, lo if lo >= 0 else None, -1)
            pieces = []
            for d in dirs:
                r0 = x3[:, asc if d == 0 else dsc, 0:nb]
                r1 = x3[:, dsc if d == 0 else asc, nb:W]
                for g in range(3):
                    ps = ppre.tile([H, ct * nb], f32, tag="ppre", name="ppret")

                    def mm0(ps=ps, l=l, d=d, g=g, r0=r0):
                        nc.tensor.matmul(ps[:], wih_t(l, d, g, 0), r0,
                                         start=True, stop=False)

                    def mm1(ps=ps, l=l, d=d, g=g, r1=r1):
                        nc.tensor.matmul(ps[:], wih_t(l, d, g, 1), r1,
                                         start=False, stop=True)

                    slot = g * gslot if g < 2 else 3 * gslot
                    dst3 = gi3[:, :, slot + d * nb: slot + (d + 1) * nb]
                    pieces.append(mm0)
                    pieces.append(mm1)
                    pieces += ev_halves(ps, dst3, bcol(l, d, g))
            return pieces

        # ---------------------- one scan step ----------------------------
        def scan_step(l, s, gi, tl, h_prev, h_out, w):
            """h_prev/h_out: (H, w) APs. w = W for layers 0-2, nb for layer 3."""
            gw = 4 * w
            gi3 = gi[:, 0:ct * gw].rearrange("p (tl g) -> p tl g", g=gw)
            # one PSUM bank per accumulation group so all three can be open
            # at once (zero-out regions are bank-granular)
            P_r = pgate.tile([H, 512], f32, tag="p_r", name="p_r")[:, 0:w]
            P_z = pgate.tile([H, 512], f32, tag="p_z", name="p_z")[:, 0:w]
            P_q = pgate.tile([H, 512], f32, tag="p_q", name="p_q")[:, 0:w]

            nc.tensor.matmul(P_r, id_sb[:], gi3[:, tl, 0:w],
                             start=True, stop=False)
            nc.tensor.matmul(P_z, id_sb[:], gi3[:, tl, w:2 * w],
                             start=True, stop=False)
            nc.tensor.matmul(P_q, id_sb[:], gi3[:, tl, 2 * w:3 * w],
                             start=True, stop=False)
            ndir = 2 if w == W else 1
            for g, Pg in enumerate((P_r, P_z, P_q)):
                for d in range(ndir):
                    hd = h_prev[:, d * nb:(d + 1) * nb]
                    nc.tensor.matmul(Pg[:, d * nb:(d + 1) * nb],
                                     whh_t(l, d, g), hd, start=False,
                                     stop=(d == ndir - 1))

            # ACT: sigma_r (chain) then sigma_zm = sigmoid(-z_pre) (off-chain)
            nc.scalar.activation(r_sb[:, 0:w], P_r, AF.Sigmoid)
            nc.scalar.activation(zm_sb[:, 0:w], P_z, AF.Sigmoid,
                                 scale=-1.0)

            # DVE: tmp = r*(q+bhn); n2 = tmp + gin
            nc.vector.tensor_tensor(tmp_sb[:, 0:w], r_sb[:, 0:w],
                                    P_q, op=ALU.mult)
            nc.vector.tensor_tensor(n2_sb[:, 0:w], tmp_sb[:, 0:w],
                                    gi3[:, tl, 3 * w:4 * w], op=ALU.add)

            # ACT: n = tanh(n2)
            nc.scalar.activation(n_sb[:, 0:w], n2_sb[:, 0:w], AF.Tanh)

            # GpSimd (during tanh, off the DVE chain): u = zm*h; zh = h - u
            nc.gpsimd.tensor_tensor(u_sb[:, 0:w], zm_sb[:, 0:w], h_prev,
                                    op=ALU.mult)
            nc.gpsimd.tensor_tensor(zh_sb[:, 0:w], h_prev, u_sb[:, 0:w],
                                    op=ALU.subtract)

            # DVE: h' = n*zm + zh
            nc.vector.tensor_tensor(nzm_sb[:, 0:w], n_sb[:, 0:w],
                                    zm_sb[:, 0:w], op=ALU.mult)
            nc.vector.tensor_tensor(h_out, nzm_sb[:, 0:w], zh_sb[:, 0:w],
                                    op=ALU.add)

        # ------------------- layer driver --------------------------------
        def run_layer(l, x_in, x_out, w, dirs=(0, 1), pre_extra=None):
            if l == 0:
                pre = lambda c, gi: pre_pieces_l0(c, gi)
            else:
                pre = lambda c, gi: pre_pieces(l, x_in, c, gi, dirs)
            gslot = W if l < 3 else nb
            for i in range(3):
                gbg = gis[i][:, 0:ct * 4 * gslot].rearrange(
                    "p (tl g) -> p tl g",
                    g=4 * gslot)[:, :, 2 * gslot:3 * gslot]
                src = dbhn012[l].rearrange("p (tl j) -> p tl j", j=W) \
                    if l < 3 else dbhn3.rearrange("p (tl j) -> p tl j", j=nb)
                nc.sync.dma_start(gbg, src)
            for piece in pre(0, gis[0]):
                piece()
            for piece in pre(1, gis[1]):
                piece()
            xo3 = None
            if x_out is not None:
                xo3 = x_out[:].rearrange("p (t w) -> p t w", w=W)
            queue = []
            for c in range(nch):
                gi = gis[c % 3]
                if c + 2 < nch:
                    queue = list(pre(c + 2, gis[(c + 2) % 3]))
                elif pre_extra is not None and c == nch - 1:
                    queue = list(pre_extra)
                    pre_extra = None
                k = max(1, (len(queue) + ct - 1) // ct) if queue else 0
                for tl in range(ct):
                    s = c * ct + tl
                    if l < 3:
                        h_prev = zeros[:, 0:w] if s == 0 else xo3[:, s - 1, :]
                        h_out = xo3[:, s, :]
                    else:
                        h_prev = zeros[:, 0:w] if s == 0 else \
                            hrot[(s - 1) % 2][:]
                        h_out = hrot[s % 2][:]
                    scan_step(l, s, gi, tl, h_prev, h_out, w)
                    for _ in range(k):
                        if queue:
                            queue.pop(0)()
            while queue:
                queue.pop(0)()

        run_layer(0, None, xA, W)
        run_layer(1, xA, xB, W)
        run_layer(2, xB, xA, W)

        # layer 3 fwd-only; its precompute tail also builds the single
        # backward-step gi (gib) from xA
        x3v = xA[:].rearrange("p (t w) -> p t w", w=W)
        bwd_pieces = []
        ps_b = ppre.tile([H, ct * nb], f32, tag="ppre", name="psb")[:, 0:3 * nb]
        for g in range(3):
            def mm0(g=g):
                nc.tensor.matmul(ps_b[:, g * nb:(g + 1) * nb],
                                 wih_t(3, 1, g, 0), x3v[:, t_len - 1, 0:nb],
                                 start=True, stop=False)

            def mm1(g=g):
                nc.tensor.matmul(ps_b[:, g * nb:(g + 1) * nb],
                                 wih_t(3, 1, g, 1), x3v[:, 0, nb:W],
                                 start=False, stop=True)

            slot = g * nb if g < 2 else 3 * nb

            def ev(g=g, slot=slot):
                nc.scalar.activation(gib[:, slot:slot + nb],
                                     ps_b[:, g * nb:(g + 1) * nb],
                                     AF.Identity, bias=bcol(3, 1, g))

            bwd_pieces += [mm0, mm1, ev]

        def bhn_b_dma():
            nc.sync.dma_start(gib[:, 2 * nb:3 * nb], dbhn3b)

        run_layer(3, xA, None, nb, dirs=(0,),
                  pre_extra=[bhn_b_dma] + bwd_pieces)
        hf = hrot[(t_len - 1) % 2][:]

        # ---- layer-3 backward single step (h0 = 0) ----
        nc.scalar.activation(r_sb[:, 0:nb], gib[:, 0:nb], AF.Sigmoid)
        nc.scalar.activation(zm_sb[:, 0:nb], gib[:, nb:2 * nb], AF.Sigmoid,
                             scale=-1.0)
        nc.vector.tensor_tensor(tmp_sb[:, 0:nb], r_sb[:, 0:nb],
                                gib[:, 2 * nb:3 * nb], op=ALU.mult)
        nc.vector.tensor_tensor(n2_sb[:, 0:nb], tmp_sb[:, 0:nb],
                                gib[:, 3 * nb:4 * nb], op=ALU.add)
        nc.scalar.activation(n_sb[:, 0:nb], n2_sb[:, 0:nb], AF.Tanh)
        nc.vector.tensor_tensor(hb_sb[:], n_sb[:, 0:nb], zm_sb[:, 0:nb],
                                op=ALU.mult)

        # ---------------- MLP head ----------------
        with tc.tile_pool(name="phead", bufs=1, space="PSUM") as php, \
                tc.tile_pool(name="shead", bufs=1) as shp:
            ph1 = php.tile([H, nb], f32)
            nc.tensor.matmul(ph1[:], w1_sb[:, 0:H], hf, start=True, stop=False)
            nc.tensor.matmul(ph1[:], w1_sb[:, H:2 * H], hb_sb[:],
                             start=False, stop=True)
            h1p = shp.tile([H, nb], f32)
            nc.scalar.activation(h1p[:], ph1[:], AF.Identity, bias=b1_sb[:])
            h1 = shp.tile([H, nb], f32)
            nc.vector.scalar_tensor_tensor(
                h1[:], h1p[:], 0.2, h1p[:],
                op0=ALU.mult, op1=ALU.max)
            po = php.tile([OUT, nb], f32)
            nc.tensor.matmul(po[:], w2_sb[:], h1[:], start=True, stop=True)
            o_sb = shp.tile([OUT, nb], f32)
            nc.scalar.activation(o_sb[:], po[:], AF.Identity, bias=b2_sb[:])
            nc.sync.dma_start(dout, o_sb[:])

    nc.compile()
    return nc


def _prep_host(raw, Wih0, Wih, Whh, bih, bhh, W1, b1, W2, b2,
               t_len=T, nb=NB, ct=CT, ncores=NCORES):
    """Host-side weight/layout prep. Returns (shared_inputs, per_core_feeds)."""
    f16 = np.float16
    Wih0 = np.asarray(Wih0, np.float32)
    Wih = np.asarray(Wih, np.float32)
    Whh = np.asarray(Whh, np.float32)
    bih = np.asarray(bih, np.float32)
    bhh = np.asarray(bhh, np.float32)
    W = 2 * nb

    w0 = np.zeros((2, 6 * H), np.float32)
    for d in range(2):
        for g in range(3):
            sl = slice(g * H, (g + 1) * H)
            w0[0, (d * 3 + g) * H:(d * 3 + g + 1) * H] = Wih0[d, sl, 0]
            bb = bih[0, d, sl] + (bhh[0, d, sl] if g < 2 else 0.0)
            w0[1, (d * 3 + g) * H:(d * 3 + g + 1) * H] = bb

    wihT = np.zeros((36, H, H), np.float32)
    for l in range(1, 4):
        for d in range(2):
            for g in range(3):
                for k in range(2):
                    i = (((l - 1) * 2 + d) * 3 + g) * 2 + k
                    wihT[i] = Wih[l - 1, d, g * H:(g + 1) * H,
                                  k * H:(k + 1) * H].T
    whhT = np.zeros((24, H, H), np.float32)
    for l in range(4):
        for d in range(2):
            for g in range(3):
                whhT[(l * 2 + d) * 3 + g] = Whh[l, d, g * H:(g + 1) * H, :].T

    bcols = np.zeros((H, 18), np.float32)
    for l in range(1, 4):
        for d in range(2):
            for g in range(3):
                sl = slice(g * H, (g + 1) * H)
                bb = bih[l, d, sl] + (bhh[l, d, sl] if g < 2 else 0.0)
                bcols[:, (l - 1) * 6 + d * 3 + g] = bb

    # bhn (= bhh n-gate) broadcast columns for the gi bhn slots
    def _bhncols(l, dirs):
        return np.concatenate([
            np.repeat(bhh[l, d, 2 * H:3 * H][:, None], nb, 1) for d in dirs],
            axis=1)

    bhn012 = np.zeros((3, H, ct * W), np.float32)
    for l in range(3):
        bhn012[l] = np.tile(_bhncols(l, (0, 1)), (1, ct))
    bhn3 = np.tile(_bhncols(3, (0,)), (1, ct))
    bhn3b = _bhncols(3, (1,))

    shared = {
        "w0": w0.astype(f16),
        "wihT": wihT.astype(f16),
        "whhT": whhT.astype(f16),
        "bcols": bcols,
        "bhn012": bhn012.astype(f16),
        "bhn3": bhn3.astype(f16),
        "bhn3b": bhn3b.astype(f16),
        "ident": np.eye(H, dtype=f16),
        "w1T": np.stack([np.asarray(W1, np.float32)[:, 0:H].T,
                         np.asarray(W1, np.float32)[:, H:2 * H].T]).astype(f16),
        "b1col": np.asarray(b1, np.float32).reshape(H, 1),
        "w2T": np.asarray(W2, np.float32).T.copy(),
        "b2col": np.asarray(b2, np.float32).reshape(OUT, 1),
    }

    x = np.asarray(raw, np.float32).reshape(-1, t_len)
    feeds = []
    for c in range(ncores):
        xs = x[c * nb:(c + 1) * nb]
        x0f = np.ones((2, t_len * nb), np.float32)
        x0f[0] = xs.T.reshape(-1)
        x0r = np.ones((2, t_len * nb), np.float32)
        x0r[0] = xs.T[::-1].reshape(-1)
        feeds.append({"x0f": x0f.astype(f16), "x0r": x0r.astype(f16)})
    return shared, feeds


def kernel(raw, Wih0, Wih, Whh, bih, bhh, W1, b1, W2, b2):
    from concourse.bass_utils import run_bass_kernel_spmd

    if "prog" not in _CACHE:
        _CACHE["prog"] = _build_program()
    nc = _CACHE["prog"]

    shared, feeds = _prep_host(raw, Wih0, Wih, Whh, bih, bhh, W1, b1, W2, b2)
    in_maps = [dict(shared, **feeds[c]) for c in range(NCORES)]
    res = run_bass_kernel_spmd(nc, in_maps, list(range(NCORES)),
                               **_CACHE.get("run_kwargs", {}))
    _CACHE["last_results"] = res
    outs = [np.asarray(res.results[c]["out"], np.float32) for c in range(NCORES)]
    full = np.concatenate(outs, axis=1)        # (8, 400)
    return np.ascontiguousarray(full.T).reshape(B, KSEQ, OUT).astype(np.float32)


# revision 5
# speedup vs baseline: 1.1338x; 1.1338x over previous
"""Trainium2 Bass kernel for 4-layer bidirectional GRU (H=128, T=200) + MLP head.

Data-parallel: 400 sequences -> 50 per core on 8 cores. Layout: 128 partitions
= hidden unit, free dim = batch slots [fwd 50 | bwd 50].

Per scan step the critical chain is:
  2 r-gate matmuls -> sigmoid(r) -> mul(r, q+bhn) -> add(gin) -> tanh
  -> mul(n, 1-z) -> add(z*h) -> h'
Off-chain: z/n matmuls, sigmoid(-z_pre) giving (1-z) directly, z*h product,
PSUM prefills via identity matmuls (biases + gi pre-loaded into the
accumulators), input-projection precompute and its PSUM->SBUF evictions
(split in halves to fit scheduling gaps), and a tiny PE-warming matmul after
each tanh to keep the PE out of its low-power state ahead of the gate
matmuls.

gi layout per step: [r(100) | z(100) | bhn(100) | gin(100)] - r/z/gin hold
input projections incl. biases (folded in the eviction's activation bias);
the bhn columns are DMA-broadcast once per layer so a single identity matmul
prefills the n-gate PSUM group with bhh_n before Whn*h accumulates onto it.
One (128,300) PSUM tile carries three independent accumulation groups
[r|z|q], so sigmoid(r) fires as soon as the two r matmuls stop.
"""

import sys

import numpy as np

_REPO = "/opt/trn_rl_repo"
if _REPO not in sys.path:
    sys.path.insert(0, _REPO)

B, KSEQ, T = 4, 100, 200
H = 128
L = 4
OUT = 8
NCORES = 8
N = B * KSEQ
NB = N // NCORES          # 50 per core
CT = 10                   # timesteps per precompute chunk
F16 = "float16"

_CACHE = {}


def _build_program(t_len=T, nb=NB, ct=CT, num_devices=NCORES):
    import concourse.bacc as bacc
    import concourse.mybir as mybir
    import concourse.tile as tile
    from contextlib import ExitStack

    f32 = mybir.dt.float32
    f16 = mybir.dt.float16
    AF = mybir.ActivationFunctionType
    ALU = mybir.AluOpType

    nch = t_len // ct
    W = 2 * nb                  # 100
    GW = 8 * nb                 # 400: gi step block [r|z|bhn|gin]
    GW3 = 4 * nb                # 200: layer-3 gi step block

    nc = bacc.Bacc("TRN2", target_bir_lowering=False, debug=False,
                   num_devices=num_devices)

    # ---- DRAM I/O ----
    dx0f = nc.dram_tensor("x0f", (2, t_len * nb), f16, kind="ExternalInput").ap()
    dx0r = nc.dram_tensor("x0r", (2, t_len * nb), f16, kind="ExternalInput").ap()
    dw0 = nc.dram_tensor("w0", (2, 6 * H), f16, kind="ExternalInput").ap()
    dwih = nc.dram_tensor("wihT", (36, H, H), f16, kind="ExternalInput").ap()
    dwhh = nc.dram_tensor("whhT", (24, H, H), f16, kind="ExternalInput").ap()
    dbcols = nc.dram_tensor("bcols", (H, 18), f32, kind="ExternalInput").ap()
    dbhn012 = nc.dram_tensor("bhn012", (3, H, ct * W), f16,
                             kind="ExternalInput").ap()
    dbhn3 = nc.dram_tensor("bhn3", (H, ct * nb), f16,
                           kind="ExternalInput").ap()
    dbhn3b = nc.dram_tensor("bhn3b", (H, nb), f16, kind="ExternalInput").ap()
    dident = nc.dram_tensor("ident", (H, H), f16, kind="ExternalInput").ap()
    dw1 = nc.dram_tensor("w1T", (2, H, H), f16, kind="ExternalInput").ap()
    db1 = nc.dram_tensor("b1col", (H, 1), f32, kind="ExternalInput").ap()
    dw2 = nc.dram_tensor("w2T", (H, OUT), f32, kind="ExternalInput").ap()
    db2 = nc.dram_tensor("b2col", (OUT, 1), f32, kind="ExternalInput").ap()
    dout = nc.dram_tensor("out", (OUT, nb), f32, kind="ExternalOutput").ap()

    with tile.TileContext(nc) as tc, ExitStack() as ctx:
        cpool = ctx.enter_context(tc.tile_pool(name="consts", bufs=1))
        pers = ctx.enter_context(tc.tile_pool(name="pers", bufs=1))
        pgate = ctx.enter_context(tc.tile_pool(name="pgate", bufs=1,
                                               space="PSUM"))
        ppre = ctx.enter_context(tc.tile_pool(name="ppre", bufs=2, space="PSUM"))

        # ---- constants / weights ----
        w0_sb = cpool.tile([2, 6 * H], f16)
        nc.sync.dma_start(w0_sb[:], dw0)
        wih_sb = cpool.tile([H, 36 * H], f16)
        nc.sync.dma_start(wih_sb[:].rearrange("p (i c) -> p i c", c=H),
                          dwih.rearrange("i p c -> p i c"))
        whh_sb = cpool.tile([H, 24 * H], f16)
        nc.sync.dma_start(whh_sb[:].rearrange("p (i c) -> p i c", c=H),
                          dwhh.rearrange("i p c -> p i c"))
        bcols_sb = cpool.tile([H, 18], f32)
        nc.sync.dma_start(bcols_sb[:], dbcols)
        id_sb = cpool.tile([H, H], f16)
        nc.sync.dma_start(id_sb[:], dident)
        w1_sb = cpool.tile([H, 2 * H], f16)
        nc.sync.dma_start(w1_sb[:].rearrange("p (i c) -> p i c", c=H),
                          dw1.rearrange("i p c -> p i c"))
        b1_sb = cpool.tile([H, 1], f32)
        nc.sync.dma_start(b1_sb[:], db1)
        w2_sb = cpool.tile([H, OUT], f32)
        nc.sync.dma_start(w2_sb[:], dw2)
        b2_sb = cpool.tile([OUT, 1], f32)
        nc.sync.dma_start(b2_sb[:], db2)

        x0f_sb = pers.tile([2, t_len * nb], f16, tag="x0f")
        nc.sync.dma_start(x0f_sb[:], dx0f)
        x0r_sb = pers.tile([2, t_len * nb], f16, tag="x0r")
        nc.sync.dma_start(x0r_sb[:], dx0r)

        # persistent state tiles
        xA = pers.tile([H, t_len * W], f16, tag="xA")
        xB = pers.tile([H, t_len * W], f16, tag="xB")
        gis = [pers.tile([H, ct * GW], f16, tag=f"gi{i}", name=f"gi{i}")
               for i in range(3)]
        r_sb = pers.tile([H, W], f16, tag="r_sb")
        zm_sb = pers.tile([H, W], f16, tag="zm_sb")
        tmp_sb = pers.tile([H, W], f16, tag="tmp_sb")
        n2_sb = pers.tile([H, W], f16, tag="n2_sb")
        n_sb = pers.tile([H, W], f16, tag="n_sb")
        u_sb = pers.tile([H, W], f16, tag="u_sb")
        zh_sb = pers.tile([H, W], f16, tag="zh_sb")
        nzm_sb = pers.tile([H, W], f16, tag="nzm_sb")
        zeros = pers.tile([H, W], f16, tag="zeros")
        hrot = [pers.tile([H, nb], f16, tag=f"hrot{i}", name=f"hrot{i}")
                for i in range(2)]
        hb_sb = pers.tile([H, nb], f16, tag="hb_sb")
        gib = pers.tile([H, GW3], f16, tag="gib")

        nc.vector.memset(zeros[:], 0.0)

        def wih_t(l, d, g, k):  # layers 1..3
            i = (((l - 1) * 2 + d) * 3 + g) * 2 + k
            return wih_sb[:, i * H:(i + 1) * H]

        def whh_t(l, d, g):
            i = (l * 2 + d) * 3 + g
            return whh_sb[:, i * H:(i + 1) * H]

        def bcol(l, d, g):
            i = (l - 1) * 6 + d * 3 + g
            return bcols_sb[:, i:i + 1]

        # ------------- precompute pieces (emitted interleaved) -------------
        def ev_halves(ps, dst3, bias):
            """Split one eviction into two halves along the chunk dim."""
            hh = ct // 2
            out = []
            for a in range(2):
                def ev(ps=ps, dst3=dst3, bias=bias, a=a):
                    src = ps[:].rearrange("p (tl n) -> p tl n", n=nb)
                    kw = {} if bias is None else {"bias": bias}
                    nc.scalar.activation(dst3[:, a * hh:(a + 1) * hh],
                                         src[:, a * hh:(a + 1) * hh],
                                         AF.Identity, **kw)
                out.append(ev)
            return out

        def pre_pieces_l0(c, gi):
            gi3 = gi[:, 0:ct * GW].rearrange("p (tl w) -> p tl w", w=GW)
            pieces = []
            for d in range(2):
                src = x0f_sb if d == 0 else x0r_sb
                rhs = src[:, c * ct * nb:(c + 1) * ct * nb]
                for g in range(3):
                    ps = ppre.tile([H, ct * nb], f32, tag="ppre", name="ppret")
                    lhsT = w0_sb[:, (d * 3 + g) * H:(d * 3 + g + 1) * H]

                    def mm(ps=ps, lhsT=lhsT, rhs=rhs):
                        nc.tensor.matmul(ps[:], lhsT, rhs, start=True,
                                         stop=True)

                    slot = g * W if g < 2 else 3 * W
                    dst3 = gi3[:, :, slot + d * nb: slot + (d + 1) * nb]
                    pieces.append(mm)
                    pieces += ev_halves(ps, dst3, None)
            return pieces

        def pre_pieces(l, x_in, c, gi, dirs=(0, 1)):
            gw = GW if l < 3 else GW3
            gslot = W if l < 3 else nb
            gi3 = gi[:, 0:ct * gw].rearrange("p (tl w) -> p tl w", w=gw)
            x3 = x_in[:].rearrange("p (t w) -> p t w", w=W)
            s0 = c * ct
            hi = t_len - 1 - s0
            lo = hi - ct
            asc = slice(s0, s0 + ct)
            dsc = slice(hi, lo if lo >= 0 else None, -1)
            pieces = []
            for d in dirs:
                r0 = x3[:, asc if d == 0 else dsc, 0:nb]
                r1 = x3[:, dsc if d == 0 else asc, nb:W]
                for g in range(3):
                    ps = ppre.tile([H, ct * nb], f32, tag="ppre", name="ppret")

                    def mm0(ps=ps, l=l, d=d, g=g, r0=r0):
                        nc.tensor.matmul(ps[:], wih_t(l, d, g, 0), r0,
                                         start=True, stop=False)

                    def mm1(ps=ps, l=l, d=d, g=g, r1=r1):
                        nc.tensor.matmul(ps[:], wih_t(l, d, g, 1), r1,
                                         start=False, stop=True)

                    slot = g * gslot if g < 2 else 3 * gslot
                    dst3 = gi3[:, :, slot + d * nb: slot + (d + 1) * nb]
                    pieces.append(mm0)
                    pieces.append(mm1)
                    pieces += ev_halves(ps, dst3, bcol(l, d, g))
            return pieces

        # ---------------------- one scan step ----------------------------
        def scan_step(l, s, gi, tl, h_prev, h_out, w):
            """h_prev/h_out: (H, w) APs. w = W for layers 0-2, nb for layer 3."""
            gw = 4 * w
            gi3 = gi[:, 0:ct * gw].rearrange("p (tl g) -> p tl g", g=gw)
            # one PSUM bank per accumulation group so all three can be open
            # at once (zero-out regions are bank-granular)
            P_r = pgate.tile([H, 512], f32, tag="p_r", name="p_r")[:, 0:w]
            P_z = pgate.tile([H, 512], f32, tag="p_z", name="p_z")[:, 0:w]
            P_q = pgate.tile([H, 512], f32, tag="p_q", name="p_q")[:, 0:w]

            nc.tensor.matmul(P_r, id_sb[:], gi3[:, tl, 0:w],
                             start=True, stop=False)
            nc.tensor.matmul(P_z, id_sb[:], gi3[:, tl, w:2 * w],
                             start=True, stop=False)
            nc.tensor.matmul(P_q, id_sb[:], gi3[:, tl, 2 * w:3 * w],
                             start=True, stop=False)
            ndir = 2 if w == W else 1
            for g, Pg in enumerate((P_r, P_z, P_q)):
                for d in range(ndir):
                    hd = h_prev[:, d * nb:(d + 1) * nb]
                    nc.tensor.matmul(Pg[:, d * nb:(d + 1) * nb],
                                     whh_t(l, d, g), hd, start=False,
                                     stop=(d == ndir - 1))

            # ACT: sigma_r (chain) then sigma_zm = sigmoid(-z_pre) (off-chain)
            nc.scalar.activation(r_sb[:, 0:w], P_r, AF.Sigmoid)
            nc.scalar.activation(zm_sb[:, 0:w], P_z, AF.Sigmoid,
                                 scale=-1.0)

            # DVE: tmp = r*(q+bhn); n2 = tmp + gin
            nc.vector.tensor_tensor(tmp_sb[:, 0:w], r_sb[:, 0:w],
                                    P_q, op=ALU.mult)
            nc.vector.tensor_tensor(n2_sb[:, 0:w], tmp_sb[:, 0:w],
                                    gi3[:, tl, 3 * w:4 * w], op=ALU.add)

            # ACT: n = tanh(n2)
            nc.scalar.activation(n_sb[:, 0:w], n2_sb[:, 0:w], AF.Tanh)

            # DVE tail, fully serial so nothing jumps the OOO queue:
            # h' = zm*(n - h) + h
            nc.vector.tensor_tensor(u_sb[:, 0:w], n_sb[:, 0:w], h_prev,
                                    op=ALU.subtract)
            nc.vector.tensor_tensor(nzm_sb[:, 0:w], zm_sb[:, 0:w],
                                    u_sb[:, 0:w], op=ALU.mult)
            nc.vector.tensor_tensor(h_out, nzm_sb[:, 0:w], h_prev,
                                    op=ALU.add)

        # ------------------- layer driver --------------------------------
        def run_layer(l, x_in, x_out, w, dirs=(0, 1), pre_extra=None):
            if l == 0:
                pre = lambda c, gi: pre_pieces_l0(c, gi)
            else:
                pre = lambda c, gi: pre_pieces(l, x_in, c, gi, dirs)
            gslot = W if l < 3 else nb
            for i in range(3):
                gbg = gis[i][:, 0:ct * 4 * gslot].rearrange(
                    "p (tl g) -> p tl g",
                    g=4 * gslot)[:, :, 2 * gslot:3 * gslot]
                src = dbhn012[l].rearrange("p (tl j) -> p tl j", j=W) \
                    if l < 3 else dbhn3.rearrange("p (tl j) -> p tl j", j=nb)
                nc.sync.dma_start(gbg, src)
            for piece in pre(0, gis[0]):
                piece()
            xo3 = None
            if x_out is not None:
                xo3 = x_out[:].rearrange("p (t w) -> p t w", w=W)
            queue = []
            for c in range(nch):
                gi = gis[c % 3]
                if c == 0:
                    queue = list(pre(1, gis[1]))
                    if nch > 2:
                        queue += list(pre(2, gis[2]))
                elif c + 2 < nch:
                    queue = list(pre(c + 2, gis[(c + 2) % 3]))
                elif pre_extra is not None and c == nch - 1:
                    queue = list(pre_extra)
                    pre_extra = None
                k = max(1, (len(queue) + ct - 1) // ct) if queue else 0
                for tl in range(ct):
                    s = c * ct + tl
                    if l < 3:
                        h_prev = zeros[:, 0:w] if s == 0 else xo3[:, s - 1, :]
                        h_out = xo3[:, s, :]
                    else:
                        h_prev = zeros[:, 0:w] if s == 0 else \
                            hrot[(s - 1) % 2][:]
                        h_out = hrot[s % 2][:]
                    scan_step(l, s, gi, tl, h_prev, h_out, w)
                    for _ in range(k):
                        if queue:
                            queue.pop(0)()
            while queue:
                queue.pop(0)()

        run_layer(0, None, xA, W)
        run_layer(1, xA, xB, W)
        run_layer(2, xB, xA, W)

        # layer 3 fwd-only; its precompute tail also builds the single
        # backward-step gi (gib) from xA
        x3v = xA[:].rearrange("p (t w) -> p t w", w=W)
        bwd_pieces = []
        ps_b = ppre.tile([H, ct * nb], f32, tag="ppre", name="psb")[:, 0:3 * nb]
        for g in range(3):
            def mm0(g=g):
                nc.tensor.matmul(ps_b[:, g * nb:(g + 1) * nb],
                                 wih_t(3, 1, g, 0), x3v[:, t_len - 1, 0:nb],
                                 start=True, stop=False)

            def mm1(g=g):
                nc.tensor.matmul(ps_b[:, g * nb:(g + 1) * nb],
                                 wih_t(3, 1, g, 1), x3v[:, 0, nb:W],
                                 start=False, stop=True)

            slot = g * nb if g < 2 else 3 * nb

            def ev(g=g, slot=slot):
                nc.scalar.activation(gib[:, slot:slot + nb],
                                     ps_b[:, g * nb:(g + 1) * nb],
                                     AF.Identity, bias=bcol(3, 1, g))

            bwd_pieces += [mm0, mm1, ev]

        def bhn_b_dma():
            nc.sync.dma_start(gib[:, 2 * nb:3 * nb], dbhn3b)

        run_layer(3, xA, None, nb, dirs=(0,),
                  pre_extra=[bhn_b_dma] + bwd_pieces)
        hf = hrot[(t_len - 1) % 2][:]

        # ---- layer-3 backward single step (h0 = 0) ----
        nc.scalar.activation(r_sb[:, 0:nb], gib[:, 0:nb], AF.Sigmoid)
        nc.scalar.activation(zm_sb[:, 0:nb], gib[:, nb:2 * nb], AF.Sigmoid,
                             scale=-1.0)
        nc.vector.tensor_tensor(tmp_sb[:, 0:nb], r_sb[:, 0:nb],
                                gib[:, 2 * nb:3 * nb], op=ALU.mult)
        nc.vector.tensor_tensor(n2_sb[:, 0:nb], tmp_sb[:, 0:nb],
                                gib[:, 3 * nb:4 * nb], op=ALU.add)
        nc.scalar.activation(n_sb[:, 0:nb], n2_sb[:, 0:nb], AF.Tanh)
        nc.vector.tensor_tensor(hb_sb[:], n_sb[:, 0:nb], zm_sb[:, 0:nb],
                                op=ALU.mult)

        # ---------------- MLP head ----------------
        with tc.tile_pool(name="phead", bufs=1, space="PSUM") as php, \
                tc.tile_pool(name="shead", bufs=1) as shp:
            ph1 = php.tile([H, nb], f32)
            nc.tensor.matmul(ph1[:], w1_sb[:, 0:H], hf, start=True, stop=False)
            nc.tensor.matmul(ph1[:], w1_sb[:, H:2 * H], hb_sb[:],
                             start=False, stop=True)
            h1p = shp.tile([H, nb], f32)
            nc.scalar.activation(h1p[:], ph1[:], AF.Identity, bias=b1_sb[:])
            h1 = shp.tile([H, nb], f32)
            nc.vector.scalar_tensor_tensor(
                h1[:], h1p[:], 0.2, h1p[:],
                op0=ALU.mult, op1=ALU.max)
            po = php.tile([OUT, nb], f32)
            nc.tensor.matmul(po[:], w2_sb[:], h1[:], start=True, stop=True)
            o_sb = shp.tile([OUT, nb], f32)
            nc.scalar.activation(o_sb[:], po[:], AF.Identity, bias=b2_sb[:])
            nc.sync.dma_start(dout, o_sb[:])

    nc.compile()
    return nc


def _prep_host(raw, Wih0, Wih, Whh, bih, bhh, W1, b1, W2, b2,
               t_len=T, nb=NB, ct=CT, ncores=NCORES):
    """Host-side weight/layout prep. Returns (shared_inputs, per_core_feeds)."""
    f16 = np.float16
    Wih0 = np.asarray(Wih0, np.float32)
    Wih = np.asarray(Wih, np.float32)
    Whh = np.asarray(Whh, np.float32)
    bih = np.asarray(bih, np.float32)
    bhh = np.asarray(bhh, np.float32)
    W = 2 * nb

    w0 = np.zeros((2, 6 * H), np.float32)
    for d in range(2):
        for g in range(3):
            sl = slice(g * H, (g + 1) * H)
            w0[0, (d * 3 + g) * H:(d * 3 + g + 1) * H] = Wih0[d, sl, 0]
            bb = bih[0, d, sl] + (bhh[0, d, sl] if g < 2 else 0.0)
            w0[1, (d * 3 + g) * H:(d * 3 + g + 1) * H] = bb

    wihT = np.zeros((36, H, H), np.float32)
    for l in range(1, 4):
        for d in range(2):
            for g in range(3):
                for k in range(2):
                    i = (((l - 1) * 2 + d) * 3 + g) * 2 + k
                    wihT[i] = Wih[l - 1, d, g * H:(g + 1) * H,
                                  k * H:(k + 1) * H].T
    whhT = np.zeros((24, H, H), np.float32)
    for l in range(4):
        for d in range(2):
            for g in range(3):
                whhT[(l * 2 + d) * 3 + g] = Whh[l, d, g * H:(g + 1) * H, :].T

    bcols = np.zeros((H, 18), np.float32)
    for l in range(1, 4):
        for d in range(2):
            for g in range(3):
                sl = slice(g * H, (g + 1) * H)
                bb = bih[l, d, sl] + (bhh[l, d, sl] if g < 2 else 0.0)
                bcols[:, (l - 1) * 6 + d * 3 + g] = bb

    # bhn (= bhh n-gate) broadcast columns for the gi bhn slots
    def _bhncols(l, dirs):
        return np.concatenate([
            np.repeat(bhh[l, d, 2 * H:3 * H][:, None], nb, 1) for d in dirs],
            axis=1)

    bhn012 = np.zeros((3, H, ct * W), np.float32)
    for l in range(3):
        bhn012[l] = np.tile(_bhncols(l, (0, 1)), (1, ct))
    bhn3 = np.tile(_bhncols(3, (0,)), (1, ct))
    bhn3b = _bhncols(3, (1,))

    shared = {
        "w0": w0.astype(f16),
        "wihT": wihT.astype(f16),
        "whhT": whhT.astype(f16),
        "bcols": bcols,
        "bhn012": bhn012.astype(f16),
        "bhn3": bhn3.astype(f16),
        "bhn3b": bhn3b.astype(f16),
        "ident": np.eye(H, dtype=f16),
        "w1T": np.stack([np.asarray(W1, np.float32)[:, 0:H].T,
                         np.asarray(W1, np.float32)[:, H:2 * H].T]).astype(f16),
        "b1col": np.asarray(b1, np.float32).reshape(H, 1),
        "w2T": np.asarray(W2, np.float32).T.copy(),
        "b2col": np.asarray(b2, np.float32).reshape(OUT, 1),
    }

    x = np.asarray(raw, np.float32).reshape(-1, t_len)
    feeds = []
    for c in range(ncores):
        xs = x[c * nb:(c + 1) * nb]
        x0f = np.ones((2, t_len * nb), np.float32)
        x0f[0] = xs.T.reshape(-1)
        x0r = np.ones((2, t_len * nb), np.float32)
        x0r[0] = xs.T[::-1].reshape(-1)
        feeds.append({"x0f": x0f.astype(f16), "x0r": x0r.astype(f16)})
    return shared, feeds


def kernel(raw, Wih0, Wih, Whh, bih, bhh, W1, b1, W2, b2):
    from concourse.bass_utils import run_bass_kernel_spmd

    if "prog" not in _CACHE:
        _CACHE["prog"] = _build_program()
    nc = _CACHE["prog"]

    shared, feeds = _prep_host(raw, Wih0, Wih, Whh, bih, bhh, W1, b1, W2, b2)
    in_maps = [dict(shared, **feeds[c]) for c in range(NCORES)]
    res = run_bass_kernel_spmd(nc, in_maps, list(range(NCORES)),
                               **_CACHE.get("run_kwargs", {}))
    _CACHE["last_results"] = res
    outs = [np.asarray(res.results[c]["out"], np.float32) for c in range(NCORES)]
    full = np.concatenate(outs, axis=1)        # (8, 400)
    return np.ascontiguousarray(full.T).reshape(B, KSEQ, OUT).astype(np.float32)


# revision 6
# speedup vs baseline: 1.1417x; 1.0069x over previous
"""Trainium2 Bass kernel for 4-layer bidirectional GRU (H=128, T=200) + MLP head.

Data-parallel: 400 sequences -> 50 per core on 8 cores. Layout: 128 partitions
= hidden unit, free dim = batch slots [fwd 50 | bwd 50].

Per scan step the critical chain is:
  2 r-gate matmuls -> sigmoid(r) -> mul(r, q+bhn) -> add(gin) -> tanh
  -> mul(n, 1-z) -> add(z*h) -> h'
Off-chain: z/n matmuls, sigmoid(-z_pre) giving (1-z) directly, z*h product,
PSUM prefills via identity matmuls (biases + gi pre-loaded into the
accumulators), input-projection precompute and its PSUM->SBUF evictions
(split in halves to fit scheduling gaps), and a tiny PE-warming matmul after
each tanh to keep the PE out of its low-power state ahead of the gate
matmuls.

gi layout per step: [r(100) | z(100) | bhn(100) | gin(100)] - r/z/gin hold
input projections incl. biases (folded in the eviction's activation bias);
the bhn columns are DMA-broadcast once per layer so a single identity matmul
prefills the n-gate PSUM group with bhh_n before Whn*h accumulates onto it.
One (128,300) PSUM tile carries three independent accumulation groups
[r|z|q], so sigmoid(r) fires as soon as the two r matmuls stop.
"""

import sys

import numpy as np

_REPO = "/opt/trn_rl_repo"
if _REPO not in sys.path:
    sys.path.insert(0, _REPO)

B, KSEQ, T = 4, 100, 200
H = 128
L = 4
OUT = 8
NCORES = 8
N = B * KSEQ
NB = N // NCORES          # 50 per core
CT = 10                   # timesteps per precompute chunk
F16 = "float16"

_CACHE = {}


def _build_program(t_len=T, nb=NB, ct=CT, num_devices=NCORES):
    import concourse.bacc as bacc
    import concourse.mybir as mybir
    import concourse.tile as tile
    from contextlib import ExitStack

    f32 = mybir.dt.float32
    f16 = mybir.dt.float16
    AF = mybir.ActivationFunctionType
    ALU = mybir.AluOpType

    nch = t_len // ct
    W = 2 * nb                  # 100
    GW = 8 * nb                 # 400: gi step block [r|z|bhn|gin]
    GW3 = 4 * nb                # 200: layer-3 gi step block

    nc = bacc.Bacc("TRN2", target_bir_lowering=False, debug=False,
                   num_devices=num_devices)

    # ---- DRAM I/O ----
    dx0f = nc.dram_tensor("x0f", (2, t_len * nb), f16, kind="ExternalInput").ap()
    dx0r = nc.dram_tensor("x0r", (2, t_len * nb), f16, kind="ExternalInput").ap()
    dw0 = nc.dram_tensor("w0", (2, 6 * H), f16, kind="ExternalInput").ap()
    dwih = nc.dram_tensor("wihT", (36, H, H), f16, kind="ExternalInput").ap()
    dwhh = nc.dram_tensor("whhT", (24, H, H), f16, kind="ExternalInput").ap()
    dbcols = nc.dram_tensor("bcols", (H, 18), f32, kind="ExternalInput").ap()
    dbhn012 = nc.dram_tensor("bhn012", (3, H, ct * W), f16,
                             kind="ExternalInput").ap()
    dbhn3 = nc.dram_tensor("bhn3", (H, ct * nb), f16,
                           kind="ExternalInput").ap()
    dbhn3b = nc.dram_tensor("bhn3b", (H, nb), f16, kind="ExternalInput").ap()
    dident = nc.dram_tensor("ident", (H, H), f16, kind="ExternalInput").ap()
    dw1 = nc.dram_tensor("w1T", (2, H, H), f16, kind="ExternalInput").ap()
    db1 = nc.dram_tensor("b1col", (H, 1), f32, kind="ExternalInput").ap()
    dw2 = nc.dram_tensor("w2T", (H, OUT), f32, kind="ExternalInput").ap()
    db2 = nc.dram_tensor("b2col", (OUT, 1), f32, kind="ExternalInput").ap()
    dout = nc.dram_tensor("out", (OUT, nb), f32, kind="ExternalOutput").ap()

    with tile.TileContext(nc) as tc, ExitStack() as ctx:
        cpool = ctx.enter_context(tc.tile_pool(name="consts", bufs=1))
        pers = ctx.enter_context(tc.tile_pool(name="pers", bufs=1))
        pgate = ctx.enter_context(tc.tile_pool(name="pgate", bufs=1,
                                               space="PSUM"))
        ppre = ctx.enter_context(tc.tile_pool(name="ppre", bufs=2, space="PSUM"))

        # ---- constants / weights ----
        w0_sb = cpool.tile([2, 6 * H], f16)
        nc.sync.dma_start(w0_sb[:], dw0)
        wih_sb = cpool.tile([H, 36 * H], f16)
        nc.sync.dma_start(wih_sb[:].rearrange("p (i c) -> p i c", c=H),
                          dwih.rearrange("i p c -> p i c"))
        whh_sb = cpool.tile([H, 24 * H], f16)
        nc.sync.dma_start(whh_sb[:].rearrange("p (i c) -> p i c", c=H),
                          dwhh.rearrange("i p c -> p i c"))
        bcols_sb = cpool.tile([H, 18], f32)
        nc.sync.dma_start(bcols_sb[:], dbcols)
        id_sb = cpool.tile([H, H], f16)
        nc.sync.dma_start(id_sb[:], dident)
        w1_sb = cpool.tile([H, 2 * H], f16)
        nc.sync.dma_start(w1_sb[:].rearrange("p (i c) -> p i c", c=H),
                          dw1.rearrange("i p c -> p i c"))
        b1_sb = cpool.tile([H, 1], f32)
        nc.sync.dma_start(b1_sb[:], db1)
        w2_sb = cpool.tile([H, OUT], f32)
        nc.sync.dma_start(w2_sb[:], dw2)
        b2_sb = cpool.tile([OUT, 1], f32)
        nc.sync.dma_start(b2_sb[:], db2)

        x0f_sb = pers.tile([2, t_len * nb], f16, tag="x0f")
        nc.sync.dma_start(x0f_sb[:], dx0f)
        x0r_sb = pers.tile([2, t_len * nb], f16, tag="x0r")
        nc.sync.dma_start(x0r_sb[:], dx0r)

        # persistent state tiles
        xA = pers.tile([H, t_len * W], f16, tag="xA")
        xB = pers.tile([H, t_len * W], f16, tag="xB")
        gis = [pers.tile([H, ct * GW], f16, tag=f"gi{i}", name=f"gi{i}")
               for i in range(3)]
        r_sb = pers.tile([H, W], f16, tag="r_sb")
        zm_sb = pers.tile([H, W], f16, tag="zm_sb")
        tmp_sb = pers.tile([H, W], f16, tag="tmp_sb")
        n2_sb = pers.tile([H, W], f16, tag="n2_sb")
        n_sb = pers.tile([H, W], f16, tag="n_sb")
        u_sb = pers.tile([H, W], f16, tag="u_sb")
        zh_sb = pers.tile([H, W], f16, tag="zh_sb")
        nzm_sb = pers.tile([H, W], f16, tag="nzm_sb")
        zeros = pers.tile([H, W], f16, tag="zeros")
        hrot = [pers.tile([H, nb], f16, tag=f"hrot{i}", name=f"hrot{i}")
                for i in range(2)]
        hb_sb = pers.tile([H, nb], f16, tag="hb_sb")
        gib = pers.tile([H, GW3], f16, tag="gib")

        nc.vector.memset(zeros[:], 0.0)

        def wih_t(l, d, g, k):  # layers 1..3
            i = (((l - 1) * 2 + d) * 3 + g) * 2 + k
            return wih_sb[:, i * H:(i + 1) * H]

        def whh_t(l, d, g):
            i = (l * 2 + d) * 3 + g
            return whh_sb[:, i * H:(i + 1) * H]

        def bcol(l, d, g):
            i = (l - 1) * 6 + d * 3 + g
            return bcols_sb[:, i:i + 1]

        # ------------- precompute pieces (emitted interleaved) -------------
        def ev_full(ps, dst3, bias):
            """One full-width eviction; fits the ACT idle window at the
            tail of a scan step."""
            def ev(ps=ps, dst3=dst3, bias=bias):
                src = ps[:].rearrange("p (tl n) -> p tl n", n=nb)
                kw = {} if bias is None else {"bias": bias}
                nc.scalar.activation(dst3, src, AF.Identity, **kw)
            return [ev]

        def pre_pieces_l0(c, gi):
            gi3 = gi[:, 0:ct * GW].rearrange("p (tl w) -> p tl w", w=GW)
            pieces = []
            for d in range(2):
                src = x0f_sb if d == 0 else x0r_sb
                rhs = src[:, c * ct * nb:(c + 1) * ct * nb]
                for g in range(3):
                    ps = ppre.tile([H, ct * nb], f32, tag="ppre", name="ppret")
                    lhsT = w0_sb[:, (d * 3 + g) * H:(d * 3 + g + 1) * H]

                    def mm(ps=ps, lhsT=lhsT, rhs=rhs):
                        nc.tensor.matmul(ps[:], lhsT, rhs, start=True,
                                         stop=True)

                    slot = g * W if g < 2 else 3 * W
                    dst3 = gi3[:, :, slot + d * nb: slot + (d + 1) * nb]
                    pieces.append(mm)
                    pieces += ev_full(ps, dst3, None)
            return pieces

        def pre_pieces(l, x_in, c, gi, dirs=(0, 1)):
            gw = GW if l < 3 else GW3
            gslot = W if l < 3 else nb
            gi3 = gi[:, 0:ct * gw].rearrange("p (tl w) -> p tl w", w=gw)
            x3 = x_in[:].rearrange("p (t w) -> p t w", w=W)
            s0 = c * ct
            hi = t_len - 1 - s0
            lo = hi - ct
            asc = slice(s0, s0 + ct)
            dsc = slice(hi, lo if lo >= 0 else None, -1)
            pieces = []
            for d in dirs:
                r0 = x3[:, asc if d == 0 else dsc, 0:nb]
                r1 = x3[:, dsc if d == 0 else asc, nb:W]
                for g in range(3):
                    ps = ppre.tile([H, ct * nb], f32, tag="ppre", name="ppret")

                    def mm0(ps=ps, l=l, d=d, g=g, r0=r0):
                        nc.tensor.matmul(ps[:], wih_t(l, d, g, 0), r0,
                                         start=True, stop=False)

                    def mm1(ps=ps, l=l, d=d, g=g, r1=r1):
                        nc.tensor.matmul(ps[:], wih_t(l, d, g, 1), r1,
                                         start=False, stop=True)

                    slot = g * gslot if g < 2 else 3 * gslot
                    dst3 = gi3[:, :, slot + d * nb: slot + (d + 1) * nb]
                    pieces.append(mm0)
                    pieces.append(mm1)
                    pieces += ev_full(ps, dst3, bcol(l, d, g))
            return pieces

        # ---------------------- one scan step ----------------------------
        def scan_step(l, s, gi, tl, h_prev, h_out, w):
            """h_prev/h_out: (H, w) APs. w = W for layers 0-2, nb for layer 3."""
            gw = 4 * w
            gi3 = gi[:, 0:ct * gw].rearrange("p (tl g) -> p tl g", g=gw)
            # one PSUM bank per accumulation group so all three can be open
            # at once (zero-out regions are bank-granular)
            P_r = pgate.tile([H, 512], f32, tag="p_r", name="p_r")[:, 0:w]
            P_z = pgate.tile([H, 512], f32, tag="p_z", name="p_z")[:, 0:w]
            P_q = pgate.tile([H, 512], f32, tag="p_q", name="p_q")[:, 0:w]

            nc.tensor.matmul(P_r, id_sb[:], gi3[:, tl, 0:w],
                             start=True, stop=False)
            nc.tensor.matmul(P_z, id_sb[:], gi3[:, tl, w:2 * w],
                             start=True, stop=False)
            nc.tensor.matmul(P_q, id_sb[:], gi3[:, tl, 2 * w:3 * w],
                             start=True, stop=False)
            ndir = 2 if w == W else 1
            for g, Pg in enumerate((P_r, P_z, P_q)):
                for d in range(ndir):
                    hd = h_prev[:, d * nb:(d + 1) * nb]
                    nc.tensor.matmul(Pg[:, d * nb:(d + 1) * nb],
                                     whh_t(l, d, g), hd, start=False,
                                     stop=(d == ndir - 1))

            # ACT: sigma_r (chain) then sigma_zm = sigmoid(-z_pre) (off-chain)
            nc.scalar.activation(r_sb[:, 0:w], P_r, AF.Sigmoid)
            nc.scalar.activation(zm_sb[:, 0:w], P_z, AF.Sigmoid,
                                 scale=-1.0)

            # DVE: tmp = r*(q+bhn); n2 = tmp + gin
            nc.vector.tensor_tensor(tmp_sb[:, 0:w], r_sb[:, 0:w],
                                    P_q, op=ALU.mult)
            nc.vector.tensor_tensor(n2_sb[:, 0:w], tmp_sb[:, 0:w],
                                    gi3[:, tl, 3 * w:4 * w], op=ALU.add)

            # ACT: n = tanh(n2)
            nc.scalar.activation(n_sb[:, 0:w], n2_sb[:, 0:w], AF.Tanh)

            # DVE tail, fully serial so nothing jumps the OOO queue:
            # h' = zm*(n - h) + h
            nc.vector.tensor_tensor(u_sb[:, 0:w], n_sb[:, 0:w], h_prev,
                                    op=ALU.subtract)
            nc.vector.tensor_tensor(nzm_sb[:, 0:w], zm_sb[:, 0:w],
                                    u_sb[:, 0:w], op=ALU.mult)
            nc.vector.tensor_tensor(h_out, nzm_sb[:, 0:w], h_prev,
                                    op=ALU.add)

        # ------------------- layer driver --------------------------------
        def run_layer(l, x_in, x_out, w, dirs=(0, 1), pre_extra=None):
            if l == 0:
                pre = lambda c, gi: pre_pieces_l0(c, gi)
            else:
                pre = lambda c, gi: pre_pieces(l, x_in, c, gi, dirs)
            gslot = W if l < 3 else nb
            for i in range(3):
                gbg = gis[i][:, 0:ct * 4 * gslot].rearrange(
                    "p (tl g) -> p tl g",
                    g=4 * gslot)[:, :, 2 * gslot:3 * gslot]
                src = dbhn012[l].rearrange("p (tl j) -> p tl j", j=W) \
                    if l < 3 else dbhn3.rearrange("p (tl j) -> p tl j", j=nb)
                nc.sync.dma_start(gbg, src)
            for piece in pre(0, gis[0]):
                piece()
            xo3 = None
            if x_out is not None:
                xo3 = x_out[:].rearrange("p (t w) -> p t w", w=W)
            queue = []
            for c in range(nch):
                gi = gis[c % 3]
                if c == 0:
                    queue = list(pre(1, gis[1]))
                    if nch > 2:
                        queue += list(pre(2, gis[2]))
                elif c + 2 < nch:
                    queue = list(pre(c + 2, gis[(c + 2) % 3]))
                elif pre_extra is not None and c == nch - 1:
                    queue = list(pre_extra)
                    pre_extra = None
                k = max(1, (len(queue) + ct - 1) // ct) if queue else 0
                for tl in range(ct):
                    s = c * ct + tl
                    if l < 3:
                        h_prev = zeros[:, 0:w] if s == 0 else xo3[:, s - 1, :]
                        h_out = xo3[:, s, :]
                    else:
                        h_prev = zeros[:, 0:w] if s == 0 else \
                            hrot[(s - 1) % 2][:]
                        h_out = hrot[s % 2][:]
                    scan_step(l, s, gi, tl, h_prev, h_out, w)
                    for _ in range(k):
                        if queue:
                            queue.pop(0)()
            while queue:
                queue.pop(0)()

        run_layer(0, None, xA, W)
        run_layer(1, xA, xB, W)
        run_layer(2, xB, xA, W)

        # layer 3 fwd-only; its precompute tail also builds the single
        # backward-step gi (gib) from xA
        x3v = xA[:].rearrange("p (t w) -> p t w", w=W)
        bwd_pieces = []
        ps_b = ppre.tile([H, ct * nb], f32, tag="ppre", name="psb")[:, 0:3 * nb]
        for g in range(3):
            def mm0(g=g):
                nc.tensor.matmul(ps_b[:, g * nb:(g + 1) * nb],
                                 wih_t(3, 1, g, 0), x3v[:, t_len - 1, 0:nb],
                                 start=True, stop=False)

            def mm1(g=g):
                nc.tensor.matmul(ps_b[:, g * nb:(g + 1) * nb],
                                 wih_t(3, 1, g, 1), x3v[:, 0, nb:W],
                                 start=False, stop=True)

            slot = g * nb if g < 2 else 3 * nb

            def ev(g=g, slot=slot):
                nc.scalar.activation(gib[:, slot:slot + nb],
                                     ps_b[:, g * nb:(g + 1) * nb],
                                     AF.Identity, bias=bcol(3, 1, g))

            bwd_pieces += [mm0, mm1, ev]

        def bhn_b_dma():
            nc.sync.dma_start(gib[:, 2 * nb:3 * nb], dbhn3b)

        run_layer(3, xA, None, nb, dirs=(0,),
                  pre_extra=[bhn_b_dma] + bwd_pieces)
        hf = hrot[(t_len - 1) % 2][:]

        # ---- layer-3 backward single step (h0 = 0) ----
        nc.scalar.activation(r_sb[:, 0:nb], gib[:, 0:nb], AF.Sigmoid)
        nc.scalar.activation(zm_sb[:, 0:nb], gib[:, nb:2 * nb], AF.Sigmoid,
                             scale=-1.0)
        nc.vector.tensor_tensor(tmp_sb[:, 0:nb], r_sb[:, 0:nb],
                                gib[:, 2 * nb:3 * nb], op=ALU.mult)
        nc.vector.tensor_tensor(n2_sb[:, 0:nb], tmp_sb[:, 0:nb],
                                gib[:, 3 * nb:4 * nb], op=ALU.add)
        nc.scalar.activation(n_sb[:, 0:nb], n2_sb[:, 0:nb], AF.Tanh)
        nc.vector.tensor_tensor(hb_sb[:], n_sb[:, 0:nb], zm_sb[:, 0:nb],
                                op=ALU.mult)

        # ---------------- MLP head ----------------
        with tc.tile_pool(name="phead", bufs=1, space="PSUM") as php, \
                tc.tile_pool(name="shead", bufs=1) as shp:
            ph1 = php.tile([H, nb], f32)
            nc.tensor.matmul(ph1[:], w1_sb[:, 0:H], hf, start=True, stop=False)
            nc.tensor.matmul(ph1[:], w1_sb[:, H:2 * H], hb_sb[:],
                             start=False, stop=True)
            h1p = shp.tile([H, nb], f32)
            nc.scalar.activation(h1p[:], ph1[:], AF.Identity, bias=b1_sb[:])
            h1 = shp.tile([H, nb], f32)
            nc.vector.scalar_tensor_tensor(
                h1[:], h1p[:], 0.2, h1p[:],
                op0=ALU.mult, op1=ALU.max)
            po = php.tile([OUT, nb], f32)
            nc.tensor.matmul(po[:], w2_sb[:], h1[:], start=True, stop=True)
            o_sb = shp.tile([OUT, nb], f32)
            nc.scalar.activation(o_sb[:], po[:], AF.Identity, bias=b2_sb[:])
            nc.sync.dma_start(dout, o_sb[:])

    nc.compile()
    return nc


def _prep_host(raw, Wih0, Wih, Whh, bih, bhh, W1, b1, W2, b2,
               t_len=T, nb=NB, ct=CT, ncores=NCORES):
    """Host-side weight/layout prep. Returns (shared_inputs, per_core_feeds)."""
    f16 = np.float16
    Wih0 = np.asarray(Wih0, np.float32)
    Wih = np.asarray(Wih, np.float32)
    Whh = np.asarray(Whh, np.float32)
    bih = np.asarray(bih, np.float32)
    bhh = np.asarray(bhh, np.float32)
    W = 2 * nb

    w0 = np.zeros((2, 6 * H), np.float32)
    for d in range(2):
        for g in range(3):
            sl = slice(g * H, (g + 1) * H)
            w0[0, (d * 3 + g) * H:(d * 3 + g + 1) * H] = Wih0[d, sl, 0]
            bb = bih[0, d, sl] + (bhh[0, d, sl] if g < 2 else 0.0)
            w0[1, (d * 3 + g) * H:(d * 3 + g + 1) * H] = bb

    wihT = np.zeros((36, H, H), np.float32)
    for l in range(1, 4):
        for d in range(2):
            for g in range(3):
                for k in range(2):
                    i = (((l - 1) * 2 + d) * 3 + g) * 2 + k
                    wihT[i] = Wih[l - 1, d, g * H:(g + 1) * H,
                                  k * H:(k + 1) * H].T
    whhT = np.zeros((24, H, H), np.float32)
    for l in range(4):
        for d in range(2):
            for g in range(3):
                whhT[(l * 2 + d) * 3 + g] = Whh[l, d, g * H:(g + 1) * H, :].T

    bcols = np.zeros((H, 18), np.float32)
    for l in range(1, 4):
        for d in range(2):
            for g in range(3):
                sl = slice(g * H, (g + 1) * H)
                bb = bih[l, d, sl] + (bhh[l, d, sl] if g < 2 else 0.0)
                bcols[:, (l - 1) * 6 + d * 3 + g] = bb

    # bhn (= bhh n-gate) broadcast columns for the gi bhn slots
    def _bhncols(l, dirs):
        return np.concatenate([
            np.repeat(bhh[l, d, 2 * H:3 * H][:, None], nb, 1) for d in dirs],
            axis=1)

    bhn012 = np.zeros((3, H, ct * W), np.float32)
    for l in range(3):
        bhn012[l] = np.tile(_bhncols(l, (0, 1)), (1, ct))
    bhn3 = np.tile(_bhncols(3, (0,)), (1, ct))
    bhn3b = _bhncols(3, (1,))

    shared = {
        "w0": w0.astype(f16),
        "wihT": wihT.astype(f16),
        "whhT": whhT.astype(f16),
        "bcols": bcols,
        "bhn012": bhn012.astype(f16),
        "bhn3": bhn3.astype(f16),
        "bhn3b": bhn3b.astype(f16),
        "ident": np.eye(H, dtype=f16),
        "w1T": np.stack([np.asarray(W1, np.float32)[:, 0:H].T,
                         np.asarray(W1, np.float32)[:, H:2 * H].T]).astype(f16),
        "b1col": np.asarray(b1, np.float32).reshape(H, 1),
        "w2T": np.asarray(W2, np.float32).T.copy(),
        "b2col": np.asarray(b2, np.float32).reshape(OUT, 1),
    }

    x = np.asarray(raw, np.float32).reshape(-1, t_len)
    feeds = []
    for c in range(ncores):
        xs = x[c * nb:(c + 1) * nb]
        x0f = np.ones((2, t_len * nb), np.float32)
        x0f[0] = xs.T.reshape(-1)
        x0r = np.ones((2, t_len * nb), np.float32)
        x0r[0] = xs.T[::-1].reshape(-1)
        feeds.append({"x0f": x0f.astype(f16), "x0r": x0r.astype(f16)})
    return shared, feeds


def kernel(raw, Wih0, Wih, Whh, bih, bhh, W1, b1, W2, b2):
    from concourse.bass_utils import run_bass_kernel_spmd

    if "prog" not in _CACHE:
        _CACHE["prog"] = _build_program()
    nc = _CACHE["prog"]

    shared, feeds = _prep_host(raw, Wih0, Wih, Whh, bih, bhh, W1, b1, W2, b2)
    in_maps = [dict(shared, **feeds[c]) for c in range(NCORES)]
    res = run_bass_kernel_spmd(nc, in_maps, list(range(NCORES)),
                               **_CACHE.get("run_kwargs", {}))
    _CACHE["last_results"] = res
    outs = [np.asarray(res.results[c]["out"], np.float32) for c in range(NCORES)]
    full = np.concatenate(outs, axis=1)        # (8, 400)
    return np.ascontiguousarray(full.T).reshape(B, KSEQ, OUT).astype(np.float32)


# revision 7
# speedup vs baseline: 1.1592x; 1.0154x over previous
"""Trainium2 Bass kernel for 4-layer bidirectional GRU (H=128, T=200) + MLP head.

Data-parallel: 400 sequences -> 50 per core on 8 cores. Layout: 128 partitions
= hidden unit, free dim = batch slots [fwd 50 | bwd 50].

Per scan step the critical chain is:
  2 r-gate matmuls -> sigmoid(r) -> mul(r, q+bhn) -> add(gin) -> tanh
  -> mul(n, 1-z) -> add(z*h) -> h'
Off-chain: z/n matmuls, sigmoid(-z_pre) giving (1-z) directly, z*h product,
PSUM prefills via identity matmuls (biases + gi pre-loaded into the
accumulators), input-projection precompute and its PSUM->SBUF evictions
(split in halves to fit scheduling gaps), and a tiny PE-warming matmul after
each tanh to keep the PE out of its low-power state ahead of the gate
matmuls.

gi layout per step: [r(100) | z(100) | bhn(100) | gin(100)] - r/z/gin hold
input projections incl. biases (folded in the eviction's activation bias);
the bhn columns are DMA-broadcast once per layer so a single identity matmul
prefills the n-gate PSUM group with bhh_n before Whn*h accumulates onto it.
One (128,300) PSUM tile carries three independent accumulation groups
[r|z|q], so sigmoid(r) fires as soon as the two r matmuls stop.
"""

import sys

import numpy as np

_REPO = "/opt/trn_rl_repo"
if _REPO not in sys.path:
    sys.path.insert(0, _REPO)

B, KSEQ, T = 4, 100, 200
H = 128
L = 4
OUT = 8
NCORES = 8
N = B * KSEQ
NB = N // NCORES          # 50 per core
CT = 10                   # timesteps per precompute chunk
F16 = "float16"

_CACHE = {}


def _build_program(t_len=T, nb=NB, ct=CT, num_devices=NCORES):
    import concourse.bacc as bacc
    import concourse.mybir as mybir
    import concourse.tile as tile
    from contextlib import ExitStack

    f32 = mybir.dt.float32
    f16 = mybir.dt.float16
    AF = mybir.ActivationFunctionType
    ALU = mybir.AluOpType

    nch = t_len // ct
    W = 2 * nb                  # 100
    GW = 8 * nb                 # 400: gi step block [r|z|bhn|gin]
    GW3 = 4 * nb                # 200: layer-3 gi step block

    nc = bacc.Bacc("TRN2", target_bir_lowering=False, debug=False,
                   num_devices=num_devices)

    # ---- DRAM I/O ----
    dx0f = nc.dram_tensor("x0f", (2, t_len * nb), f16, kind="ExternalInput").ap()
    dx0r = nc.dram_tensor("x0r", (2, t_len * nb), f16, kind="ExternalInput").ap()
    dw0 = nc.dram_tensor("w0", (2, 6 * H), f16, kind="ExternalInput").ap()
    dwih = nc.dram_tensor("wihT", (36, H, H), f16, kind="ExternalInput").ap()
    dwhh = nc.dram_tensor("whhT", (24, H, H), f16, kind="ExternalInput").ap()
    dbcols = nc.dram_tensor("bcols", (H, 18), f32, kind="ExternalInput").ap()
    dbhn012 = nc.dram_tensor("bhn012", (3, H, ct * W), f16,
                             kind="ExternalInput").ap()
    dbhn3 = nc.dram_tensor("bhn3", (H, ct * nb), f16,
                           kind="ExternalInput").ap()
    dbhn3b = nc.dram_tensor("bhn3b", (H, nb), f16, kind="ExternalInput").ap()
    dident = nc.dram_tensor("ident", (H, H), f16, kind="ExternalInput").ap()
    dw1 = nc.dram_tensor("w1T", (2, H, H), f16, kind="ExternalInput").ap()
    db1 = nc.dram_tensor("b1col", (H, 1), f32, kind="ExternalInput").ap()
    dw2 = nc.dram_tensor("w2T", (H, OUT), f32, kind="ExternalInput").ap()
    db2 = nc.dram_tensor("b2col", (OUT, 1), f32, kind="ExternalInput").ap()
    dout = nc.dram_tensor("out", (OUT, nb), f32, kind="ExternalOutput").ap()

    with tile.TileContext(nc) as tc, ExitStack() as ctx:
        cpool = ctx.enter_context(tc.tile_pool(name="consts", bufs=1))
        pers = ctx.enter_context(tc.tile_pool(name="pers", bufs=1))
        pgate = ctx.enter_context(tc.tile_pool(name="pgate", bufs=1,
                                               space="PSUM"))
        ppre = ctx.enter_context(tc.tile_pool(name="ppre", bufs=2, space="PSUM"))
        ppad = ctx.enter_context(tc.tile_pool(name="ppad", bufs=1,
                                              space="PSUM"))

        # ---- constants / weights ----
        w0_sb = cpool.tile([2, 6 * H], f16)
        nc.sync.dma_start(w0_sb[:], dw0)
        wih_sb = cpool.tile([H, 36 * H], f16)
        nc.sync.dma_start(wih_sb[:].rearrange("p (i c) -> p i c", c=H),
                          dwih.rearrange("i p c -> p i c"))
        whh_sb = cpool.tile([H, 24 * H], f16)
        nc.sync.dma_start(whh_sb[:].rearrange("p (i c) -> p i c", c=H),
                          dwhh.rearrange("i p c -> p i c"))
        bcols_sb = cpool.tile([H, 18], f32)
        nc.sync.dma_start(bcols_sb[:], dbcols)
        id_sb = cpool.tile([H, H], f16)
        nc.sync.dma_start(id_sb[:], dident)
        w1_sb = cpool.tile([H, 2 * H], f16)
        nc.sync.dma_start(w1_sb[:].rearrange("p (i c) -> p i c", c=H),
                          dw1.rearrange("i p c -> p i c"))
        b1_sb = cpool.tile([H, 1], f32)
        nc.sync.dma_start(b1_sb[:], db1)
        w2_sb = cpool.tile([H, OUT], f32)
        nc.sync.dma_start(w2_sb[:], dw2)
        b2_sb = cpool.tile([OUT, 1], f32)
        nc.sync.dma_start(b2_sb[:], db2)

        x0f_sb = pers.tile([2, t_len * nb], f16, tag="x0f")
        nc.sync.dma_start(x0f_sb[:], dx0f)
        x0r_sb = pers.tile([2, t_len * nb], f16, tag="x0r")
        nc.sync.dma_start(x0r_sb[:], dx0r)

        # persistent state tiles
        xA = pers.tile([H, t_len * W], f16, tag="xA")
        xB = pers.tile([H, t_len * W], f16, tag="xB")
        gis = [pers.tile([H, ct * GW], f16, tag=f"gi{i}", name=f"gi{i}")
               for i in range(3)]
        r_sb = pers.tile([H, W], f16, tag="r_sb")
        zm_sb = pers.tile([H, W], f16, tag="zm_sb")
        tmp_sb = pers.tile([H, W], f16, tag="tmp_sb")
        n2_sb = pers.tile([H, W], f16, tag="n2_sb")
        n_sb = pers.tile([H, W], f16, tag="n_sb")
        u_sb = pers.tile([H, W], f16, tag="u_sb")
        zh_sb = pers.tile([H, W], f16, tag="zh_sb")
        nzm_sb = pers.tile([H, W], f16, tag="nzm_sb")
        zeros = pers.tile([H, W], f16, tag="zeros")
        hrot = [pers.tile([H, nb], f16, tag=f"hrot{i}", name=f"hrot{i}")
                for i in range(2)]
        hb_sb = pers.tile([H, nb], f16, tag="hb_sb")
        gib = pers.tile([H, GW3], f16, tag="gib")

        nc.vector.memset(zeros[:], 0.0)

        def wih_t(l, d, g, k):  # layers 1..3
            i = (((l - 1) * 2 + d) * 3 + g) * 2 + k
            return wih_sb[:, i * H:(i + 1) * H]

        def whh_t(l, d, g):
            i = (l * 2 + d) * 3 + g
            return whh_sb[:, i * H:(i + 1) * H]

        def bcol(l, d, g):
            i = (l - 1) * 6 + d * 3 + g
            return bcols_sb[:, i:i + 1]

        # ------------- precompute pieces (emitted interleaved) -------------
        def ev_full(ps, dst3, bias):
            """One full-width eviction; fits the ACT idle window at the
            tail of a scan step."""
            def ev(ps=ps, dst3=dst3, bias=bias):
                src = ps[:].rearrange("p (tl n) -> p tl n", n=nb)
                kw = {} if bias is None else {"bias": bias}
                nc.scalar.activation(dst3, src, AF.Identity, **kw)
            return [ev]

        def pre_pieces_l0(c, gi):
            gi3 = gi[:, 0:ct * GW].rearrange("p (tl w) -> p tl w", w=GW)
            pieces = []
            for d in range(2):
                src = x0f_sb if d == 0 else x0r_sb
                rhs = src[:, c * ct * nb:(c + 1) * ct * nb]
                for g in range(3):
                    ps = ppre.tile([H, ct * nb], f32, tag="ppre", name="ppret")
                    lhsT = w0_sb[:, (d * 3 + g) * H:(d * 3 + g + 1) * H]

                    def mm(ps=ps, lhsT=lhsT, rhs=rhs):
                        nc.tensor.matmul(ps[:], lhsT, rhs, start=True,
                                         stop=True)

                    slot = g * W if g < 2 else 3 * W
                    dst3 = gi3[:, :, slot + d * nb: slot + (d + 1) * nb]
                    pieces.append(mm)
                    pieces += ev_full(ps, dst3, None)
            return pieces

        def pre_pieces(l, x_in, c, gi, dirs=(0, 1)):
            gw = GW if l < 3 else GW3
            gslot = W if l < 3 else nb
            gi3 = gi[:, 0:ct * gw].rearrange("p (tl w) -> p tl w", w=gw)
            x3 = x_in[:].rearrange("p (t w) -> p t w", w=W)
            s0 = c * ct
            hi = t_len - 1 - s0
            lo = hi - ct
            asc = slice(s0, s0 + ct)
            dsc = slice(hi, lo if lo >= 0 else None, -1)
            pieces = []
            for d in dirs:
                r0 = x3[:, asc if d == 0 else dsc, 0:nb]
                r1 = x3[:, dsc if d == 0 else asc, nb:W]
                for g in range(3):
                    ps = ppre.tile([H, ct * nb], f32, tag="ppre", name="ppret")

                    def mm0(ps=ps, l=l, d=d, g=g, r0=r0):
                        nc.tensor.matmul(ps[:], wih_t(l, d, g, 0), r0,
                                         start=True, stop=False)

                    def mm1(ps=ps, l=l, d=d, g=g, r1=r1):
                        nc.tensor.matmul(ps[:], wih_t(l, d, g, 1), r1,
                                         start=False, stop=True)

                    slot = g * gslot if g < 2 else 3 * gslot
                    dst3 = gi3[:, :, slot + d * nb: slot + (d + 1) * nb]
                    pieces.append(mm0)
                    pieces.append(mm1)
                    pieces += ev_full(ps, dst3, bcol(l, d, g))
            return pieces

        # ---------------------- one scan step ----------------------------
        def scan_step(l, s, gi, tl, h_prev, h_out, w):
            """h_prev/h_out: (H, w) APs. w = W for layers 0-2, nb for layer 3."""
            gw = 4 * w
            gi3 = gi[:, 0:ct * gw].rearrange("p (tl g) -> p tl g", g=gw)
            # one PSUM bank per accumulation group so all three can be open
            # at once (zero-out regions are bank-granular)
            P_r = pgate.tile([H, 512], f32, tag="p_r", name="p_r")[:, 0:w]
            P_z = pgate.tile([H, 512], f32, tag="p_z", name="p_z")[:, 0:w]
            P_q = pgate.tile([H, 512], f32, tag="p_q", name="p_q")[:, 0:w]

            nc.tensor.matmul(P_r, id_sb[:], gi3[:, tl, 0:w],
                             start=True, stop=False)
            nc.tensor.matmul(P_z, id_sb[:], gi3[:, tl, w:2 * w],
                             start=True, stop=False)
            nc.tensor.matmul(P_q, id_sb[:], gi3[:, tl, 2 * w:3 * w],
                             start=True, stop=False)
            ndir = 2 if w == W else 1
            for g, Pg in enumerate((P_r, P_z, P_q)):
                for d in range(ndir):
                    hd = h_prev[:, d * nb:(d + 1) * nb]
                    nc.tensor.matmul(Pg[:, d * nb:(d + 1) * nb],
                                     whh_t(l, d, g), hd, start=False,
                                     stop=(d == ndir - 1))

            # ACT: sigma_r (chain) then sigma_zm = sigmoid(-z_pre) (off-chain)
            nc.scalar.activation(r_sb[:, 0:w], P_r, AF.Sigmoid)
            nc.scalar.activation(zm_sb[:, 0:w], P_z, AF.Sigmoid,
                                 scale=-1.0)

            # GpSimd: u = zm*h. Completes mid-step, so the DVE zh below only
            # becomes ready AFTER n2 has dispatched (no OOO queue-jump).
            nc.gpsimd.tensor_tensor(u_sb[:, 0:w], zm_sb[:, 0:w], h_prev,
                                    op=ALU.mult)

            # DVE: tmp = r*(q+bhn); n2 = tmp + gin; zh = h - u (idle window)
            nc.vector.tensor_tensor(tmp_sb[:, 0:w], r_sb[:, 0:w],
                                    P_q, op=ALU.mult)
            nc.vector.tensor_tensor(n2_sb[:, 0:w], tmp_sb[:, 0:w],
                                    gi3[:, tl, 3 * w:4 * w], op=ALU.add)
            nc.vector.tensor_tensor(zh_sb[:, 0:w], h_prev, u_sb[:, 0:w],
                                    op=ALU.subtract)

            # ACT: n = tanh(n2)
            nc.scalar.activation(n_sb[:, 0:w], n2_sb[:, 0:w], AF.Tanh)

            # DVE tail: h' = zm*n + zh
            nc.vector.tensor_tensor(nzm_sb[:, 0:w], zm_sb[:, 0:w],
                                    n_sb[:, 0:w], op=ALU.mult)
            nc.vector.tensor_tensor(h_out, nzm_sb[:, 0:w], zh_sb[:, 0:w],
                                    op=ALU.add)

            # dummy wide matmuls keep the PE streaming through the pointwise
            # phase - sustained PE load holds the core's fast power-state
            # (measured: every engine's instructions run 25-40% faster)
            for j in range(2):
                pp = ppad.tile([H, 512], f32, tag="ppad", name="ppadt")
                nc.tensor.matmul(pp[:, 0:500], id_sb[:],
                                 wih_sb[:, j * 500:(j + 1) * 500],
                                 start=True, stop=True)

        # ------------------- layer driver --------------------------------
        def run_layer(l, x_in, x_out, w, dirs=(0, 1), pre_extra=None):
            if l == 0:
                pre = lambda c, gi: pre_pieces_l0(c, gi)
            else:
                pre = lambda c, gi: pre_pieces(l, x_in, c, gi, dirs)
            gslot = W if l < 3 else nb
            for i in range(3):
                gbg = gis[i][:, 0:ct * 4 * gslot].rearrange(
                    "p (tl g) -> p tl g",
                    g=4 * gslot)[:, :, 2 * gslot:3 * gslot]
                src = dbhn012[l].rearrange("p (tl j) -> p tl j", j=W) \
                    if l < 3 else dbhn3.rearrange("p (tl j) -> p tl j", j=nb)
                nc.sync.dma_start(gbg, src)
            for piece in pre(0, gis[0]):
                piece()
            xo3 = None
            if x_out is not None:
                xo3 = x_out[:].rearrange("p (t w) -> p t w", w=W)
            queue = []
            for c in range(nch):
                gi = gis[c % 3]
                if c == 0:
                    queue = list(pre(1, gis[1]))
                    if nch > 2:
                        queue += list(pre(2, gis[2]))
                elif c + 2 < nch:
                    queue = list(pre(c + 2, gis[(c + 2) % 3]))
                elif pre_extra is not None and c == nch - 1:
                    queue = list(pre_extra)
                    pre_extra = None
                k = max(1, (len(queue) + ct - 1) // ct) if queue else 0
                for tl in range(ct):
                    s = c * ct + tl
                    if l < 3:
                        h_prev = zeros[:, 0:w] if s == 0 else xo3[:, s - 1, :]
                        h_out = xo3[:, s, :]
                    else:
                        h_prev = zeros[:, 0:w] if s == 0 else \
                            hrot[(s - 1) % 2][:]
                        h_out = hrot[s % 2][:]
                    scan_step(l, s, gi, tl, h_prev, h_out, w)
                    for _ in range(k):
                        if queue:
                            queue.pop(0)()
            while queue:
                queue.pop(0)()

        run_layer(0, None, xA, W)
        run_layer(1, xA, xB, W)
        run_layer(2, xB, xA, W)

        # layer 3 fwd-only; its precompute tail also builds the single
        # backward-step gi (gib) from xA
        x3v = xA[:].rearrange("p (t w) -> p t w", w=W)
        bwd_pieces = []
        ps_b = ppre.tile([H, ct * nb], f32, tag="ppre", name="psb")[:, 0:3 * nb]
        for g in range(3):
            def mm0(g=g):
                nc.tensor.matmul(ps_b[:, g * nb:(g + 1) * nb],
                                 wih_t(3, 1, g, 0), x3v[:, t_len - 1, 0:nb],
                                 start=True, stop=False)

            def mm1(g=g):
                nc.tensor.matmul(ps_b[:, g * nb:(g + 1) * nb],
                                 wih_t(3, 1, g, 1), x3v[:, 0, nb:W],
                                 start=False, stop=True)

            slot = g * nb if g < 2 else 3 * nb

            def ev(g=g, slot=slot):
                nc.scalar.activation(gib[:, slot:slot + nb],
                                     ps_b[:, g * nb:(g + 1) * nb],
                                     AF.Identity, bias=bcol(3, 1, g))

            bwd_pieces += [mm0, mm1, ev]

        def bhn_b_dma():
            nc.sync.dma_start(gib[:, 2 * nb:3 * nb], dbhn3b)

        run_layer(3, xA, None, nb, dirs=(0,),
                  pre_extra=[bhn_b_dma] + bwd_pieces)
        hf = hrot[(t_len - 1) % 2][:]

        # ---- layer-3 backward single step (h0 = 0) ----
        nc.scalar.activation(r_sb[:, 0:nb], gib[:, 0:nb], AF.Sigmoid)
        nc.scalar.activation(zm_sb[:, 0:nb], gib[:, nb:2 * nb], AF.Sigmoid,
                             scale=-1.0)
        nc.vector.tensor_tensor(tmp_sb[:, 0:nb], r_sb[:, 0:nb],
                                gib[:, 2 * nb:3 * nb], op=ALU.mult)
        nc.vector.tensor_tensor(n2_sb[:, 0:nb], tmp_sb[:, 0:nb],
                                gib[:, 3 * nb:4 * nb], op=ALU.add)
        nc.scalar.activation(n_sb[:, 0:nb], n2_sb[:, 0:nb], AF.Tanh)
        nc.vector.tensor_tensor(hb_sb[:], n_sb[:, 0:nb], zm_sb[:, 0:nb],
                                op=ALU.mult)

        # ---------------- MLP head ----------------
        with tc.tile_pool(name="phead", bufs=1, space="PSUM") as php, \
                tc.tile_pool(name="shead", bufs=1) as shp:
            ph1 = php.tile([H, nb], f32)
            nc.tensor.matmul(ph1[:], w1_sb[:, 0:H], hf, start=True, stop=False)
            nc.tensor.matmul(ph1[:], w1_sb[:, H:2 * H], hb_sb[:],
                             start=False, stop=True)
            h1p = shp.tile([H, nb], f32)
            nc.scalar.activation(h1p[:], ph1[:], AF.Identity, bias=b1_sb[:])
            h1 = shp.tile([H, nb], f32)
            nc.vector.scalar_tensor_tensor(
                h1[:], h1p[:], 0.2, h1p[:],
                op0=ALU.mult, op1=ALU.max)
            po = php.tile([OUT, nb], f32)
            nc.tensor.matmul(po[:], w2_sb[:], h1[:], start=True, stop=True)
            o_sb = shp.tile([OUT, nb], f32)
            nc.scalar.activation(o_sb[:], po[:], AF.Identity, bias=b2_sb[:])
            nc.sync.dma_start(dout, o_sb[:])

    nc.compile()
    return nc


def _prep_host(raw, Wih0, Wih, Whh, bih, bhh, W1, b1, W2, b2,
               t_len=T, nb=NB, ct=CT, ncores=NCORES):
    """Host-side weight/layout prep. Returns (shared_inputs, per_core_feeds)."""
    f16 = np.float16
    Wih0 = np.asarray(Wih0, np.float32)
    Wih = np.asarray(Wih, np.float32)
    Whh = np.asarray(Whh, np.float32)
    bih = np.asarray(bih, np.float32)
    bhh = np.asarray(bhh, np.float32)
    W = 2 * nb

    w0 = np.zeros((2, 6 * H), np.float32)
    for d in range(2):
        for g in range(3):
            sl = slice(g * H, (g + 1) * H)
            w0[0, (d * 3 + g) * H:(d * 3 + g + 1) * H] = Wih0[d, sl, 0]
            bb = bih[0, d, sl] + (bhh[0, d, sl] if g < 2 else 0.0)
            w0[1, (d * 3 + g) * H:(d * 3 + g + 1) * H] = bb

    wihT = np.zeros((36, H, H), np.float32)
    for l in range(1, 4):
        for d in range(2):
            for g in range(3):
                for k in range(2):
                    i = (((l - 1) * 2 + d) * 3 + g) * 2 + k
                    wihT[i] = Wih[l - 1, d, g * H:(g + 1) * H,
                                  k * H:(k + 1) * H].T
    whhT = np.zeros((24, H, H), np.float32)
    for l in range(4):
        for d in range(2):
            for g in range(3):
                whhT[(l * 2 + d) * 3 + g] = Whh[l, d, g * H:(g + 1) * H, :].T

    bcols = np.zeros((H, 18), np.float32)
    for l in range(1, 4):
        for d in range(2):
            for g in range(3):
                sl = slice(g * H, (g + 1) * H)
                bb = bih[l, d, sl] + (bhh[l, d, sl] if g < 2 else 0.0)
                bcols[:, (l - 1) * 6 + d * 3 + g] = bb

    # bhn (= bhh n-gate) broadcast columns for the gi bhn slots
    def _bhncols(l, dirs):
        return np.concatenate([
            np.repeat(bhh[l, d, 2 * H:3 * H][:, None], nb, 1) for d in dirs],
            axis=1)

    bhn012 = np.zeros((3, H, ct * W), np.float32)
    for l in range(3):
        bhn012[l] = np.tile(_bhncols(l, (0, 1)), (1, ct))
    bhn3 = np.tile(_bhncols(3, (0,)), (1, ct))
    bhn3b = _bhncols(3, (1,))

    shared = {
        "w0": w0.astype(f16),
        "wihT": wihT.astype(f16),
        "whhT": whhT.astype(f16),
        "bcols": bcols,
        "bhn012": bhn012.astype(f16),
        "bhn3": bhn3.astype(f16),
        "bhn3b": bhn3b.astype(f16),
        "ident": np.eye(H, dtype=f16),
        "w1T": np.stack([np.asarray(W1, np.float32)[:, 0:H].T,
                         np.asarray(W1, np.float32)[:, H:2 * H].T]).astype(f16),
        "b1col": np.asarray(b1, np.float32).reshape(H, 1),
        "w2T": np.asarray(W2, np.float32).T.copy(),
        "b2col": np.asarray(b2, np.float32).reshape(OUT, 1),
    }

    x = np.asarray(raw, np.float32).reshape(-1, t_len)
    feeds = []
    for c in range(ncores):
        xs = x[c * nb:(c + 1) * nb]
        x0f = np.ones((2, t_len * nb), np.float32)
        x0f[0] = xs.T.reshape(-1)
        x0r = np.ones((2, t_len * nb), np.float32)
        x0r[0] = xs.T[::-1].reshape(-1)
        feeds.append({"x0f": x0f.astype(f16), "x0r": x0r.astype(f16)})
    return shared, feeds


def kernel(raw, Wih0, Wih, Whh, bih, bhh, W1, b1, W2, b2):
    from concourse.bass_utils import run_bass_kernel_spmd

    if "prog" not in _CACHE:
        _CACHE["prog"] = _build_program()
    nc = _CACHE["prog"]

    shared, feeds = _prep_host(raw, Wih0, Wih, Whh, bih, bhh, W1, b1, W2, b2)
    in_maps = [dict(shared, **feeds[c]) for c in range(NCORES)]
    res = run_bass_kernel_spmd(nc, in_maps, list(range(NCORES)),
                               **_CACHE.get("run_kwargs", {}))
    _CACHE["last_results"] = res
    outs = [np.asarray(res.results[c]["out"], np.float32) for c in range(NCORES)]
    full = np.concatenate(outs, axis=1)        # (8, 400)
    return np.ascontiguousarray(full.T).reshape(B, KSEQ, OUT).astype(np.float32)
